# revision 10
# baseline (speedup 1.0000x reference)
"""DDiT attention block on 8 trn2 NeuronCores.

Sharding: data-parallel over batch (cores 0-3 -> batch 0, cores 4-7 ->
batch 1) x tensor-parallel over heads (4 heads/core, Megatron-style:
W_qkv row-sharded, W_out column-sharded). Per-head y shards are
AllGather'd within each 4-core group in t-halves as soon as each half
finishes, and the output projection accumulates per-head chunks, so
collectives overlap the remaining attention compute. Each core produces
a 256-column slice of the output, assembled on the host.

Per core (1 batch, 4 heads, T=2048, C=1024, D=64):
  qT,kT = Wqk_shard @ x.T        [512, 2048]   (features on partitions)
  v     = x @ Wv_shard.T         [2048, 256]   (seq on partitions) + ones col
  ST_h  = exp((kT_h.T @ qT_h)/8) [2048s, 2048t] streamed in [128,512] tiles,
          the two heads of a pair computed as concurrent row-tiled matmuls
  ytaug_h = [v_h | 1].T @ ST_h   [65, 2048]    row 64 = softmax denominator l
  y_h   = ytaug_h[:64] * recip(l)  (DVE reciprocal + gpsimd partition bcast)
  AllGather y_h halves over the group -> [256, 1024] x2
  out  += gathered.T @ wo_h      (wo host-permuted to the gathered row order)

v2 notes (vs the 357us baseline): the PE spent the whole baseline run at
the 1.2 GHz mid p-state because the HAM clock gate never saw 3.4us of
continuous matmul work (32 serialized input-DMA issues at the start,
exp-gated micro-gaps in steady state).  This version batches input DMAs
(5 instructions), issues the projection as one dense back-to-back burst
to warm the PE, keeps matmuls flowing through every AllGather window by
finalizing heads in t-halves, and moves softmax-denominator work off the
ACT engine (DVE reciprocal + gpsimd broadcast) since ACT exp is the
steady-state pacer.
"""

import os
import sys

sys.path.insert(0, "/opt/trn_rl_repo")

import numpy as np
import ml_dtypes

import concourse.bass as bass
import concourse.mybir as mybir
import concourse.tile as tile_mod
from concourse.tile import TileContext
from concourse.vector_clock import ScopedClock

F32 = mybir.dt.float32
BF16 = mybir.dt.bfloat16
AF = mybir.ActivationFunctionType

B, T, C = 2, 2048, 1024
H, D = 16, 64
NCORES = 8
GROUP = 4            # cores per batch group (tensor-parallel degree)
HPC = H // GROUP     # heads per core = 4
FQK = 2 * HPC * D    # 512 qk features per core
FV = HPC * D         # 256 v features per core
KT = C // 128        # 8 contraction tiles
TT128 = T // 128     # 16 seq tiles of 128
TT512 = T // 512     # 4 seq tiles of 512
T2 = T // 2          # 1024: finalize/AllGather half
REPLICA_GROUPS = [[0, 1, 2, 3], [4, 5, 6, 7]]

# ---------------------------------------------------------------------------
# walrus workarounds: this build rejects >1 sync-wait command per
# instruction. Move excess waits onto standalone event-semaphore nops on the
# same engine queue (equivalent to raw-bass wait_ge + op).
# ---------------------------------------------------------------------------
_WAITSPLIT_CTR = [0]


def _split_excess_waits(nc: bass.Bass, limit: int = 1) -> int:
    moved = 0
    for f in nc.m.functions:
        for bb in f.blocks:
            insts = bb.instructions
            i = 0
            while i < len(insts):
                inst = insts[i]
                si = inst.sync_info
                if si is not None and si.on_wait and len(si.on_wait) > limit:
                    waits = list(si.on_wait)
                    si.on_wait = waits[:limit]
                    for w in waits[limit:]:
                        _WAITSPLIT_CTR[0] += 1
                        moved += 1
                        ev = mybir.InstEventSemaphore(
                            name=f"I-waitsplit-{_WAITSPLIT_CTR[0]}",
                            engine=inst.engine,
                            ins=[],
                            outs=[],
                            sync_info=mybir.SyncInfo(on_wait=[w], on_update=[]),
                        )
                        insts.insert(i, ev)
                        i += 1
                i += 1
    return moved


def _patched_drain_and_barrier(self, tick_clock, wait_clock):
    nc = self.nc
    nop0 = nc.sync.nop(nofuse=True, hint="tile_exit_waits")
    wait_clock.add_sem_waits(nop0.ins, ScopedClock({None: tick_clock.global_clock}))
    nc.sync.drain()
    nc.all_engine_barrier()
    assert self.sems is not None
    popped = nc._tile_sem_poison_stack.pop()
    assert popped is self._sem_poison
    nc.clear_and_free_semaphores(list(self.sems.allocated().values()))
    nc.all_engine_barrier()


def _install_ntff_shim():
    """Provide antenv.axon_hooks (absent in this image) so trace=True can
    reach the libaxon NTFF profiler."""
    import types

    if "antenv.axon_hooks" in sys.modules:
        return
    hook = None
    try:
        sys.path.insert(0, "/root/.axon_site")
        from trn_agent_boot.trn_boot import _ntff_profile_via_ctypes

        so_path = "/opt/axon/libaxon_pjrt.so"
        if os.path.exists(so_path):
            hook = _ntff_profile_via_ctypes(so_path)
    except Exception:
        hook = None
    mod = types.ModuleType("antenv.axon_hooks")
    mod.get_axon_ntff_profile_hook = lambda: hook
    mod.set_axon_ntff_profile_hook = lambda h: None
    sys.modules["antenv.axon_hooks"] = mod


tile_mod.TileContext._drain_and_barrier = _patched_drain_and_barrier
_install_ntff_shim()


# ---------------------------------------------------------------------------
# device program (identical on all 8 cores; per-core data differs)
# ---------------------------------------------------------------------------
def _build() -> bass.Bass:
    nc = bass.Bass(trn_type="TRN2", target_bir_lowering=False, num_devices=NCORES)

    xT = nc.dram_tensor("xT", [C, T], BF16, kind="ExternalInput")
    wqk = nc.dram_tensor("wqk", [C, FQK], BF16, kind="ExternalInput")
    wv = nc.dram_tensor("wv", [C, FV], BF16, kind="ExternalInput")
    wo = nc.dram_tensor("wo", [FV, HPC * FV], BF16, kind="ExternalInput")
    out = nc.dram_tensor("out", [T, FV], F32, kind="ExternalOutput")

    cc_in = [
        [nc.dram_tensor(f"cc_in{h}_{x2}", [D, T2], BF16) for x2 in range(2)]
        for h in range(HPC)
    ]
    cc_out = [
        [nc.dram_tensor(f"cc_out{h}_{x2}", [GROUP * D, T2], BF16) for x2 in range(2)]
        for h in range(HPC)
    ]

    out_v = out.rearrange("(tt p) f -> tt p f", p=128)

    with TileContext(nc) as tc:
        with (
            tc.tile_pool(name="pw", bufs=1) as pw,
            tc.tile_pool(name="px", bufs=1) as px,
            tc.tile_pool(name="pqkv", bufs=1) as pqkv,
            tc.tile_pool(name="pacc", bufs=1) as pacc,
            tc.tile_pool(name="patt", bufs=2) as patt,
            tc.tile_pool(name="pst", bufs=6) as pst,
            tc.tile_pool(name="pfin", bufs=2) as pfin,
            tc.tile_pool(name="pych", bufs=4) as pych,
            tc.tile_pool(name="ps_big", bufs=2, space="PSUM") as ps_big,
            tc.tile_pool(name="ps_sm", bufs=2, space="PSUM") as ps_sm,
            tc.tile_pool(name="ps_yt", bufs=1, space="PSUM") as ps_yt,
        ):
            # ---- batched input DMAs (5 issues total) -----------------------
            wqk_sb = pw.tile([128, KT * FQK], BF16, name="wqk_sb")
            wv_sb = pw.tile([128, KT * FV], BF16, name="wv_sb")
            wo_sb = pw.tile([128, 2 * HPC * FV], BF16, name="wo_sb")
            x_sb = [px.tile([128, 4 * T], BF16, name=f"x{i}") for i in range(2)]

            nc.sync.dma_start(
                out=wqk_sb[:].rearrange("p (kt f) -> p kt f", f=FQK),
                in_=wqk.rearrange("(kt p) f -> p kt f", p=128),
            )
            xT_v = xT.rearrange("(kt p) t -> p kt t", p=128)
            for i in range(2):
                nc.sync.dma_start(
                    out=x_sb[i][:].rearrange("p (kt t) -> p kt t", t=T),
                    in_=xT_v[:, 4 * i : 4 * (i + 1), :],
                )
            nc.sync.dma_start(
                out=wv_sb[:].rearrange("p (kt f) -> p kt f", f=FV),
                in_=wv.rearrange("(kt p) f -> p kt f", p=128),
            )
            nc.sync.dma_start(
                out=wo_sb[:].rearrange("p (i f) -> p i f", f=HPC * FV),
                in_=wo.rearrange("(i p) f -> p i f", p=128),
            )

            def xs(k, lo, hi):
                return x_sb[k // 4][:, (k % 4) * T + lo : (k % 4) * T + hi]

            ones1 = pw.tile([1, 64], BF16, name="ones1")
            nc.vector.memset(ones1[:], 1.0)

            # persistent activation tiles
            qk_sb = [pqkv.tile([128, T], BF16, name=f"qk{m}") for m in range(4)]
            v_sb = [
                pqkv.tile([128, HPC * (D + 1)], BF16, name=f"v{t}")
                for t in range(TT128)
            ]
            out_acc = [pacc.tile([128, FV], F32, name=f"oacc{t}") for t in range(TT128)]

            # ---- q01 / k01 projection: one dense back-to-back burst -------
            # (wqk dram columns are [q 0..255 | k 0..255] so m=0 -> q heads
            # 01, m=2 -> k heads 01, m=1 -> q23, m=3 -> k23)
            for dst, m in ((0, 0), (1, 2)):
                for q in range(TT512):
                    ps = ps_sm.tile([128, 512], F32, name="sm_ps", tag="sm")
                    for k in range(KT):
                        nc.tensor.matmul(
                            ps[:],
                            wqk_sb[:, k * FQK + 128 * m : k * FQK + 128 * (m + 1)],
                            xs(k, 512 * q, 512 * (q + 1)),
                            start=(k == 0),
                            stop=(k == KT - 1),
                        )
                    nc.vector.tensor_copy(
                        out=qk_sb[dst][:, 512 * q : 512 * (q + 1)], in_=ps[:]
                    )

            # ---- v projection ([t-part, f-col] + ones cols) ----------------
            for t in range(TT128):
                ps = ps_sm.tile([128, 512], F32, name="sm_ps", tag="sm")
                for k in range(KT):
                    nc.tensor.matmul(
                        ps[:, 0:FV],
                        xs(k, 128 * t, 128 * (t + 1)),
                        wv_sb[:, k * FV : (k + 1) * FV],
                        start=(k == 0),
                        stop=(k == KT - 1),
                    )
                vt = v_sb[t].rearrange("p (h g) -> p h g", g=D + 1)
                nc.vector.memset(v_sb[t][:], 1.0)
                nc.vector.tensor_copy(
                    out=vt[:, :, 0:D],
                    in_=ps[:, 0:FV].rearrange("p (h f) -> p h f", f=D),
                )

            # ---- q23 / k23 projection (512-wide, fills attention slack) ----
            for dst, m in ((2, 1), (3, 3)):
                for q in range(TT512):
                    ps = ps_sm.tile([128, 512], F32, name="sm_ps", tag="sm")
                    for k in range(KT):
                        nc.tensor.matmul(
                            ps[:],
                            wqk_sb[:, k * FQK + 128 * m : k * FQK + 128 * (m + 1)],
                            xs(k, 512 * q, 512 * (q + 1)),
                            start=(k == 0),
                            stop=(k == KT - 1),
                        )
                    nc.vector.tensor_copy(
                        out=qk_sb[dst][:, 512 * q : 512 * (q + 1)], in_=ps[:]
                    )

            # ---- attention + per-half finalize / AllGather / out-proj ------
            for j in range(HPC // 2):  # head pairs (local heads 2j, 2j+1)
                qtile = 2 * j
                ktile = 2 * j + 1
                yt_sb = {
                    hi: patt.tile([D + 1, T], F32, name=f"yt_sb{hi}", tag=f"yt_sb{hi}")
                    for hi in range(2)
                }
                for n in range(TT512):
                    tsl = slice(512 * n, 512 * (n + 1))
                    yt_ps = {
                        hi: ps_yt.tile([D + 1, 512], F32, name=f"yt{hi}", tag=f"yt{hi}")
                        for hi in range(2)
                    }
                    for s in range(TT128):
                        ssl = slice(128 * s, 128 * (s + 1))
                        st_ps = ps_big.tile([128, T2], F32, name="st_ps", tag="big")
                        for hi in range(2):
                            psl = slice(64 * hi, 64 * (hi + 1))
                            nc.tensor.matmul(
                                st_ps[:, 512 * hi : 512 * (hi + 1)],
                                qk_sb[ktile][psl, ssl],
                                qk_sb[qtile][psl, tsl],
                                start=True,
                                stop=True,
                            )
                        ste = pst.tile([128, T2], BF16, name="st_e")
                        nc.scalar.activation(
                            out=ste[:], in_=st_ps[:], func=AF.Exp, scale=0.125
                        )
                        for hi in range(2):
                            h = 2 * j + hi
                            vsl = slice((D + 1) * h, (D + 1) * (h + 1))
                            nc.tensor.matmul(
                                yt_ps[hi][:],
                                v_sb[s][:, vsl],
                                ste[:, 512 * hi : 512 * (hi + 1)],
                                start=(s == 0),
                                stop=(s == TT128 - 1),
                            )
                    for hi in range(2):
                        nc.vector.tensor_copy(out=yt_sb[hi][:, tsl], in_=yt_ps[hi][:])

                    if n % 2 == 0:
                        continue
                    # finalize the completed t-half of both heads: normalize,
                    # AllGather, accumulate this half's out-projection chunk
                    x2 = n // 2
                    hsl = slice(T2 * x2, T2 * (x2 + 1))
                    for hi in range(2):
                        h = 2 * j + hi
                        r_h = pfin.tile([1, T2], BF16, name="r_h", tag="r_h")
                        with nc.allow_low_precision(reason="softmax recip in f16"):
                            nc.vector.reciprocal(
                                out=r_h[:], in_=yt_sb[hi][D : D + 1, hsl]
                            )
                        ytn = pfin.tile([D, T2], BF16, name="ytn", tag="ytn")
                        for q in range(2):
                            qsl = slice(512 * q, 512 * (q + 1))
                            rb = ps_sm.tile([128, 512], F32, name="sm_ps", tag="sm")
                            nc.tensor.matmul(
                                rb[0:D, :],
                                ones1[:],
                                r_h[:, qsl],
                                start=True,
                                stop=True,
                            )
                            nc.vector.tensor_tensor(
                                out=ytn[:, qsl],
                                in0=yt_sb[hi][0:D, T2 * x2 + 512 * q : T2 * x2 + 512 * (q + 1)],
                                in1=rb[0:D, :],
                                op=mybir.AluOpType.mult,
                            )
                        nc.sync.dma_start(out=cc_in[h][x2][:], in_=ytn[:])
                        nc.gpsimd.collective_compute(
                            "AllGather",
                            mybir.AluOpType.bypass,
                            ins=[cc_in[h][x2][:]],
                            outs=[cc_out[h][x2][:]],
                            replica_groups=REPLICA_GROUPS,
                        )
                        ych = [
                            pych.tile([128, T2], BF16, name=f"ych{i}", tag=f"ych{i}")
                            for i in range(2)
                        ]
                        for i in range(2):
                            nc.sync.dma_start(
                                out=ych[i][:],
                                in_=cc_out[h][x2][128 * i : 128 * (i + 1), :],
                            )
                        for tt in range(8):
                            t = 8 * x2 + tt
                            op = ps_sm.tile([128, 512], F32, name="sm_ps", tag="sm")
                            for i in range(2):
                                nc.tensor.matmul(
                                    op[:, 0:FV],
                                    ych[i][:, 128 * tt : 128 * (tt + 1)],
                                    wo_sb[:, HPC * FV * i + FV * h : HPC * FV * i + FV * (h + 1)],
                                    start=(i == 0),
                                    stop=(i == 1),
                                )
                            if h == 0:
                                nc.vector.tensor_copy(
                                    out=out_acc[t][:], in_=op[:, 0:FV]
                                )
                            else:
                                nc.vector.tensor_tensor(
                                    out=out_acc[t][:],
                                    in0=out_acc[t][:],
                                    in1=op[:, 0:FV],
                                    op=mybir.AluOpType.add,
                                )
                            if h == HPC - 1:
                                nc.sync.dma_start(out=out_v[t], in_=out_acc[t][:])

    _split_excess_waits(nc)
    return nc


_NC_CACHE = []
LAST_RESULTS = None


def kernel(**inputs: np.ndarray) -> np.ndarray:
    global LAST_RESULTS
    from concourse.bass_utils import run_bass_kernel_spmd

    x = np.asarray(inputs["x"], dtype=np.float32)
    W_qkv = np.asarray(inputs["W_qkv"], dtype=np.float32)
    W_out = np.asarray(inputs["W_out"], dtype=np.float32)

    in_maps = []
    for c in range(NCORES):
        g, r = divmod(c, GROUP)
        q_rows = W_qkv[FV * r : FV * (r + 1)]
        k_rows = W_qkv[C + FV * r : C + FV * (r + 1)]
        v_rows = W_qkv[2 * C + FV * r : 2 * C + FV * (r + 1)]
        im = {
            "xT": np.ascontiguousarray(x[g].T).astype(ml_dtypes.bfloat16),
            "wqk": np.ascontiguousarray(
                np.concatenate([q_rows, k_rows], axis=0).T
            ).astype(ml_dtypes.bfloat16),
            "wv": np.ascontiguousarray(v_rows.T).astype(ml_dtypes.bfloat16),
        }
        wo_slice = W_out[FV * r : FV * (r + 1)]  # [256 o, 1024 c]
        wo_heads = []
        for h in range(HPC):
            cols = np.concatenate(
                [np.arange(64 * (GROUP * rr + h), 64 * (GROUP * rr + h) + 64)
                 for rr in range(GROUP)]
            )
            wo_heads.append(wo_slice[:, cols].T)  # [256 c-rows, 256 o]
        im["wo"] = np.ascontiguousarray(np.concatenate(wo_heads, axis=1)).astype(
            ml_dtypes.bfloat16
        )
        in_maps.append(im)

    if not _NC_CACHE:
        _NC_CACHE.append(_build())
    nc = _NC_CACHE[0]

    trace = os.environ.get("KERNEL_TRACE", "0") == "1"
    trace_cores = None
    if trace:
        tc_env = os.environ.get("KERNEL_TRACE_CORES", "0")
        trace_cores = [int(t) for t in tc_env.split(",")]
    res = run_bass_kernel_spmd(
        nc,
        in_maps,
        core_ids=list(range(NCORES)),
        trace=trace,
        trace_cores=trace_cores,
    )
    LAST_RESULTS = res

    out = np.empty((B, T, C), dtype=np.float32)
    for c in range(NCORES):
        g, r = divmod(c, GROUP)
        out[g, :, FV * r : FV * (r + 1)] = res.results[c]["out"]
    return out


# revision 12
# speedup vs baseline: 1.0017x; 1.0017x over previous
"""DDiT attention block on 8 trn2 NeuronCores.

Sharding: data-parallel over batch (cores 0-3 -> batch 0, cores 4-7 ->
batch 1) x tensor-parallel over heads (4 heads/core, Megatron-style:
W_qkv row-sharded, W_out column-sharded). Per-head y shards are
AllGather'd within each 4-core group in t-halves as soon as each half
finishes, and the output projection accumulates per-head chunks, so
collectives overlap the remaining attention compute. Each core produces
a 256-column slice of the output, assembled on the host.

Per core (1 batch, 4 heads, T=2048, C=1024, D=64):
  qT,kT = Wqk_shard @ x.T        [512, 2048]   (features on partitions)
  v     = x @ Wv_shard.T         [2048, 256]   (seq on partitions) + ones col
  ST_h  = exp((kT_h.T @ qT_h)/8) [2048s, 2048t] streamed in [128,512] tiles,
          the two heads of a pair computed as concurrent row-tiled matmuls
  ytaug_h = [v_h | 1].T @ ST_h   [65, 2048]    row 64 = softmax denominator l
  y_h   = ytaug_h[:64] * recip(l)  (DVE reciprocal + gpsimd partition bcast)
  AllGather y_h halves over the group -> [256, 1024] x2
  out  += gathered.T @ wo_h      (wo host-permuted to the gathered row order)

v2 notes (vs the 357us baseline): the PE spent the whole baseline run at
the 1.2 GHz mid p-state because the HAM clock gate never saw 3.4us of
continuous matmul work (32 serialized input-DMA issues at the start,
exp-gated micro-gaps in steady state).  This version batches input DMAs
(5 instructions), issues the projection as one dense back-to-back burst
to warm the PE, keeps matmuls flowing through every AllGather window by
finalizing heads in t-halves, and moves softmax-denominator work off the
ACT engine (DVE reciprocal + gpsimd broadcast) since ACT exp is the
steady-state pacer.
"""

import os
import sys

sys.path.insert(0, "/opt/trn_rl_repo")

import numpy as np
import ml_dtypes

import concourse.bass as bass
import concourse.mybir as mybir
import concourse.tile as tile_mod
from concourse.tile import TileContext
from concourse.vector_clock import ScopedClock

F32 = mybir.dt.float32
BF16 = mybir.dt.bfloat16
AF = mybir.ActivationFunctionType

B, T, C = 2, 2048, 1024
H, D = 16, 64
NCORES = 8
GROUP = 4            # cores per batch group (tensor-parallel degree)
HPC = H // GROUP     # heads per core = 4
FQK = 2 * HPC * D    # 512 qk features per core
FV = HPC * D         # 256 v features per core
KT = C // 128        # 8 contraction tiles
TT128 = T // 128     # 16 seq tiles of 128
TT512 = T // 512     # 4 seq tiles of 512
T2 = T // 2          # 1024: finalize/AllGather half
REPLICA_GROUPS = [[0, 1, 2, 3], [4, 5, 6, 7]]

# ---------------------------------------------------------------------------
# walrus workarounds: this build rejects >1 sync-wait command per
# instruction. Move excess waits onto standalone event-semaphore nops on the
# same engine queue (equivalent to raw-bass wait_ge + op).
# ---------------------------------------------------------------------------
_WAITSPLIT_CTR = [0]


def _split_excess_waits(nc: bass.Bass, limit: int = 1) -> int:
    moved = 0
    for f in nc.m.functions:
        for bb in f.blocks:
            insts = bb.instructions
            i = 0
            while i < len(insts):
                inst = insts[i]
                si = inst.sync_info
                if si is not None and si.on_wait and len(si.on_wait) > limit:
                    waits = list(si.on_wait)
                    si.on_wait = waits[:limit]
                    for w in waits[limit:]:
                        _WAITSPLIT_CTR[0] += 1
                        moved += 1
                        ev = mybir.InstEventSemaphore(
                            name=f"I-waitsplit-{_WAITSPLIT_CTR[0]}",
                            engine=inst.engine,
                            ins=[],
                            outs=[],
                            sync_info=mybir.SyncInfo(on_wait=[w], on_update=[]),
                        )
                        insts.insert(i, ev)
                        i += 1
                i += 1
    return moved


def _patched_drain_and_barrier(self, tick_clock, wait_clock):
    nc = self.nc
    nop0 = nc.sync.nop(nofuse=True, hint="tile_exit_waits")
    wait_clock.add_sem_waits(nop0.ins, ScopedClock({None: tick_clock.global_clock}))
    nc.sync.drain()
    nc.all_engine_barrier()
    assert self.sems is not None
    popped = nc._tile_sem_poison_stack.pop()
    assert popped is self._sem_poison
    nc.clear_and_free_semaphores(list(self.sems.allocated().values()))
    nc.all_engine_barrier()


def _install_ntff_shim():
    """Provide antenv.axon_hooks (absent in this image) so trace=True can
    reach the libaxon NTFF profiler."""
    import types

    if "antenv.axon_hooks" in sys.modules:
        return
    hook = None
    try:
        sys.path.insert(0, "/root/.axon_site")
        from trn_agent_boot.trn_boot import _ntff_profile_via_ctypes

        so_path = "/opt/axon/libaxon_pjrt.so"
        if os.path.exists(so_path):
            hook = _ntff_profile_via_ctypes(so_path)
    except Exception:
        hook = None
    mod = types.ModuleType("antenv.axon_hooks")
    mod.get_axon_ntff_profile_hook = lambda: hook
    mod.set_axon_ntff_profile_hook = lambda h: None
    sys.modules["antenv.axon_hooks"] = mod


tile_mod.TileContext._drain_and_barrier = _patched_drain_and_barrier
_install_ntff_shim()


# ---------------------------------------------------------------------------
# device program (identical on all 8 cores; per-core data differs)
# ---------------------------------------------------------------------------
def _build() -> bass.Bass:
    nc = bass.Bass(trn_type="TRN2", target_bir_lowering=False, num_devices=NCORES)

    xT = nc.dram_tensor("xT", [C, T], BF16, kind="ExternalInput")
    wqk = nc.dram_tensor("wqk", [C, FQK], BF16, kind="ExternalInput")
    wv = nc.dram_tensor("wv", [C, FV], BF16, kind="ExternalInput")
    wo = nc.dram_tensor("wo", [FV, HPC * FV], BF16, kind="ExternalInput")
    out = nc.dram_tensor("out", [T, FV], F32, kind="ExternalOutput")

    cc_in = [
        [nc.dram_tensor(f"cc_in{h}_{x2}", [D, T2], BF16) for x2 in range(2)]
        for h in range(HPC)
    ]
    cc_out = [
        [nc.dram_tensor(f"cc_out{h}_{x2}", [GROUP * D, T2], BF16) for x2 in range(2)]
        for h in range(HPC)
    ]

    out_v = out.rearrange("(tt p) f -> tt p f", p=128)

    with TileContext(nc) as tc:
        with (
            tc.tile_pool(name="pw", bufs=1) as pw,
            tc.tile_pool(name="px", bufs=1) as px,
            tc.tile_pool(name="pqkv", bufs=1) as pqkv,
            tc.tile_pool(name="pacc", bufs=1) as pacc,
            tc.tile_pool(name="patt", bufs=2) as patt,
            tc.tile_pool(name="pst", bufs=6) as pst,
            tc.tile_pool(name="pfin", bufs=2) as pfin,
            tc.tile_pool(name="pych", bufs=4) as pych,
            tc.tile_pool(name="ps_big", bufs=2, space="PSUM") as ps_big,
            tc.tile_pool(name="ps_sm", bufs=2, space="PSUM") as ps_sm,
            tc.tile_pool(name="ps_yt", bufs=1, space="PSUM") as ps_yt,
        ):
            # ---- batched input DMAs (5 issues total) -----------------------
            wqk_sb = pw.tile([128, KT * FQK], BF16, name="wqk_sb")
            wv_sb = pw.tile([128, KT * FV], BF16, name="wv_sb")
            wo_sb = pw.tile([128, 2 * HPC * FV], BF16, name="wo_sb")
            x_sb = [px.tile([128, 4 * T], BF16, name=f"x{i}") for i in range(2)]

            nc.sync.dma_start(
                out=wqk_sb[:].rearrange("p (kt f) -> p kt f", f=FQK),
                in_=wqk.rearrange("(kt p) f -> p kt f", p=128),
            )
            xT_v = xT.rearrange("(kt p) t -> p kt t", p=128)
            for i in range(2):
                nc.sync.dma_start(
                    out=x_sb[i][:].rearrange("p (kt t) -> p kt t", t=T),
                    in_=xT_v[:, 4 * i : 4 * (i + 1), :],
                )
            nc.sync.dma_start(
                out=wv_sb[:].rearrange("p (kt f) -> p kt f", f=FV),
                in_=wv.rearrange("(kt p) f -> p kt f", p=128),
            )
            nc.sync.dma_start(
                out=wo_sb[:].rearrange("p (i f) -> p i f", f=HPC * FV),
                in_=wo.rearrange("(i p) f -> p i f", p=128),
            )

            def xs(k, lo, hi):
                return x_sb[k // 4][:, (k % 4) * T + lo : (k % 4) * T + hi]

            ones1 = pw.tile([1, 64], BF16, name="ones1")
            nc.vector.memset(ones1[:], 1.0)

            # persistent activation tiles
            qk_sb = [pqkv.tile([128, T], BF16, name=f"qk{m}") for m in range(4)]
            v_sb = [
                pqkv.tile([128, HPC * (D + 1)], BF16, name=f"v{t}")
                for t in range(TT128)
            ]
            out_acc = [pacc.tile([128, FV], F32, name=f"oacc{t}") for t in range(TT128)]

            # ---- q01 / k01 projection: one dense back-to-back burst -------
            # (wqk dram columns are [q 0..255 | k 0..255] so m=0 -> q heads
            # 01, m=2 -> k heads 01, m=1 -> q23, m=3 -> k23)
            for dst, m in ((0, 0), (1, 2)):
                for q in range(TT512):
                    ps = ps_sm.tile([128, 512], F32, name="sm_ps", tag="sm")
                    for k in range(KT):
                        nc.tensor.matmul(
                            ps[:],
                            wqk_sb[:, k * FQK + 128 * m : k * FQK + 128 * (m + 1)],
                            xs(k, 512 * q, 512 * (q + 1)),
                            start=(k == 0),
                            stop=(k == KT - 1),
                        )
                    nc.vector.tensor_copy(
                        out=qk_sb[dst][:, 512 * q : 512 * (q + 1)], in_=ps[:]
                    )

            # ---- v projection ([t-part, f-col] + ones cols) ----------------
            for t in range(TT128):
                ps = ps_sm.tile([128, 512], F32, name="sm_ps", tag="sm")
                for k in range(KT):
                    nc.tensor.matmul(
                        ps[:, 0:FV],
                        xs(k, 128 * t, 128 * (t + 1)),
                        wv_sb[:, k * FV : (k + 1) * FV],
                        start=(k == 0),
                        stop=(k == KT - 1),
                    )
                vt = v_sb[t].rearrange("p (h g) -> p h g", g=D + 1)
                nc.vector.memset(v_sb[t][:], 1.0)
                nc.vector.tensor_copy(
                    out=vt[:, :, 0:D],
                    in_=ps[:, 0:FV].rearrange("p (h f) -> p h f", f=D),
                )

            # ---- q23 / k23 projection (512-wide, fills attention slack) ----
            for dst, m in ((2, 1), (3, 3)):
                for q in range(TT512):
                    ps = ps_sm.tile([128, 512], F32, name="sm_ps", tag="sm")
                    for k in range(KT):
                        nc.tensor.matmul(
                            ps[:],
                            wqk_sb[:, k * FQK + 128 * m : k * FQK + 128 * (m + 1)],
                            xs(k, 512 * q, 512 * (q + 1)),
                            start=(k == 0),
                            stop=(k == KT - 1),
                        )
                    nc.vector.tensor_copy(
                        out=qk_sb[dst][:, 512 * q : 512 * (q + 1)], in_=ps[:]
                    )

            # ---- attention + per-half finalize / AllGather / out-proj ------
            # The out-projection for a finished t-half is EMITTED one
            # half-phase later than its AllGather is issued: per-engine
            # instruction order is program order, so matmuls that wait on a
            # collective roundtrip must sit behind ~35us of attention work or
            # they head-of-line block the PE queue.
            deferred_op = []

            def emit_outproj(j, x2, ych):
                for hi in range(2):
                    h = 2 * j + hi
                    for tt in range(8):
                        t = 8 * x2 + tt
                        op = ps_sm.tile([128, 512], F32, name="sm_ps", tag="sm")
                        for i in range(2):
                            nc.tensor.matmul(
                                op[:, 0:FV],
                                ych[hi][i][:, 128 * tt : 128 * (tt + 1)],
                                wo_sb[:, HPC * FV * i + FV * h : HPC * FV * i + FV * (h + 1)],
                                start=(i == 0),
                                stop=(i == 1),
                            )
                        if h == 0:
                            nc.vector.tensor_copy(out=out_acc[t][:], in_=op[:, 0:FV])
                        else:
                            nc.vector.tensor_tensor(
                                out=out_acc[t][:],
                                in0=out_acc[t][:],
                                in1=op[:, 0:FV],
                                op=mybir.AluOpType.add,
                            )
                        if h == HPC - 1:
                            nc.sync.dma_start(out=out_v[t], in_=out_acc[t][:])

            for j in range(HPC // 2):  # head pairs (local heads 2j, 2j+1)
                qtile = 2 * j
                ktile = 2 * j + 1
                yt_sb = {
                    hi: patt.tile([D + 1, T], F32, name=f"yt_sb{hi}", tag=f"yt_sb{hi}")
                    for hi in range(2)
                }
                for n in range(TT512):
                    tsl = slice(512 * n, 512 * (n + 1))
                    yt_ps = {
                        hi: ps_yt.tile([D + 1, 512], F32, name=f"yt{hi}", tag=f"yt{hi}")
                        for hi in range(2)
                    }
                    for s in range(TT128):
                        ssl = slice(128 * s, 128 * (s + 1))
                        st_ps = ps_big.tile([128, T2], F32, name="st_ps", tag="big")
                        for hi in range(2):
                            psl = slice(64 * hi, 64 * (hi + 1))
                            nc.tensor.matmul(
                                st_ps[:, 512 * hi : 512 * (hi + 1)],
                                qk_sb[ktile][psl, ssl],
                                qk_sb[qtile][psl, tsl],
                                start=True,
                                stop=True,
                            )
                        ste = pst.tile([128, T2], BF16, name="st_e")
                        nc.scalar.activation(
                            out=ste[:], in_=st_ps[:], func=AF.Exp, scale=0.125
                        )
                        for hi in range(2):
                            h = 2 * j + hi
                            vsl = slice((D + 1) * h, (D + 1) * (h + 1))
                            nc.tensor.matmul(
                                yt_ps[hi][:],
                                v_sb[s][:, vsl],
                                ste[:, 512 * hi : 512 * (hi + 1)],
                                start=(s == 0),
                                stop=(s == TT128 - 1),
                            )
                    for hi in range(2):
                        nc.vector.tensor_copy(out=yt_sb[hi][:, tsl], in_=yt_ps[hi][:])

                    if n % 2 == 0:
                        continue
                    # finalize the completed t-half of both heads: normalize
                    # and issue the AllGather now; defer the out-projection
                    x2 = n // 2
                    hsl = slice(T2 * x2, T2 * (x2 + 1))
                    ych = {}
                    for hi in range(2):
                        h = 2 * j + hi
                        r_h = pfin.tile([1, T2], BF16, name="r_h", tag="r_h")
                        with nc.allow_low_precision(reason="softmax recip in f16"):
                            nc.vector.reciprocal(
                                out=r_h[:], in_=yt_sb[hi][D : D + 1, hsl]
                            )
                        ytn = pfin.tile([D, T2], BF16, name="ytn", tag="ytn")
                        for q in range(2):
                            qsl = slice(512 * q, 512 * (q + 1))
                            rb = ps_sm.tile([128, 512], F32, name="sm_ps", tag="sm")
                            nc.tensor.matmul(
                                rb[0:D, :],
                                ones1[:],
                                r_h[:, qsl],
                                start=True,
                                stop=True,
                            )
                            nc.vector.tensor_tensor(
                                out=ytn[:, qsl],
                                in0=yt_sb[hi][0:D, T2 * x2 + 512 * q : T2 * x2 + 512 * (q + 1)],
                                in1=rb[0:D, :],
                                op=mybir.AluOpType.mult,
                            )
                        nc.sync.dma_start(out=cc_in[h][x2][:], in_=ytn[:])
                        nc.gpsimd.collective_compute(
                            "AllGather",
                            mybir.AluOpType.bypass,
                            ins=[cc_in[h][x2][:]],
                            outs=[cc_out[h][x2][:]],
                            replica_groups=REPLICA_GROUPS,
                        )
                        ych[hi] = [
                            pych.tile([128, T2], BF16, name=f"ych{i}", tag=f"ych{i}")
                            for i in range(2)
                        ]
                        for i in range(2):
                            nc.sync.dma_start(
                                out=ych[hi][i][:],
                                in_=cc_out[h][x2][128 * i : 128 * (i + 1), :],
                            )
                    deferred_op.append((j, x2, ych))
                    if len(deferred_op) > 1:
                        emit_outproj(*deferred_op.pop(0))
            while deferred_op:
                emit_outproj(*deferred_op.pop(0))

    _split_excess_waits(nc)
    return nc


_NC_CACHE = []
LAST_RESULTS = None


def kernel(**inputs: np.ndarray) -> np.ndarray:
    global LAST_RESULTS
    from concourse.bass_utils import run_bass_kernel_spmd

    x = np.asarray(inputs["x"], dtype=np.float32)
    W_qkv = np.asarray(inputs["W_qkv"], dtype=np.float32)
    W_out = np.asarray(inputs["W_out"], dtype=np.float32)

    in_maps = []
    for c in range(NCORES):
        g, r = divmod(c, GROUP)
        q_rows = W_qkv[FV * r : FV * (r + 1)]
        k_rows = W_qkv[C + FV * r : C + FV * (r + 1)]
        v_rows = W_qkv[2 * C + FV * r : 2 * C + FV * (r + 1)]
        im = {
            "xT": np.ascontiguousarray(x[g].T).astype(ml_dtypes.bfloat16),
            "wqk": np.ascontiguousarray(
                np.concatenate([q_rows, k_rows], axis=0).T
            ).astype(ml_dtypes.bfloat16),
            "wv": np.ascontiguousarray(v_rows.T).astype(ml_dtypes.bfloat16),
        }
        wo_slice = W_out[FV * r : FV * (r + 1)]  # [256 o, 1024 c]
        wo_heads = []
        for h in range(HPC):
            cols = np.concatenate(
                [np.arange(64 * (GROUP * rr + h), 64 * (GROUP * rr + h) + 64)
                 for rr in range(GROUP)]
            )
            wo_heads.append(wo_slice[:, cols].T)  # [256 c-rows, 256 o]
        im["wo"] = np.ascontiguousarray(np.concatenate(wo_heads, axis=1)).astype(
            ml_dtypes.bfloat16
        )
        in_maps.append(im)

    if not _NC_CACHE:
        _NC_CACHE.append(_build())
    nc = _NC_CACHE[0]

    trace = os.environ.get("KERNEL_TRACE", "0") == "1"
    trace_cores = None
    if trace:
        tc_env = os.environ.get("KERNEL_TRACE_CORES", "0")
        trace_cores = [int(t) for t in tc_env.split(",")]
    res = run_bass_kernel_spmd(
        nc,
        in_maps,
        core_ids=list(range(NCORES)),
        trace=trace,
        trace_cores=trace_cores,
    )
    LAST_RESULTS = res

    out = np.empty((B, T, C), dtype=np.float32)
    for c in range(NCORES):
        g, r = divmod(c, GROUP)
        out[g, :, FV * r : FV * (r + 1)] = res.results[c]["out"]
    return out


# revision 16
# speedup vs baseline: 1.2247x; 1.2226x over previous
"""DDiT attention block on 8 trn2 NeuronCores.

Sharding: data-parallel over batch (cores 0-3 -> batch 0, cores 4-7 ->
batch 1) x tensor-parallel over heads (4 heads/core, Megatron-style:
W_qkv row-sharded, W_out column-sharded). Per-head y shards are
AllGather'd within each 4-core group in t-halves as soon as each half
finishes, and the output projection accumulates per-head chunks, so
collectives overlap the remaining attention compute. Each core produces
a 256-column slice of the output, assembled on the host.

Per core (1 batch, 4 heads, T=2048, C=1024, D=64):
  qT,kT = Wqk_shard @ x.T        [512, 2048]   (features on partitions)
  v     = x @ Wv_shard.T         [2048, 256]   (seq on partitions) + ones col
  ST_h  = exp((kT_h.T @ qT_h)/8) [2048s, 2048t] streamed in [128,512] tiles,
          the two heads of a pair computed as concurrent row-tiled matmuls
  ytaug_h = [v_h | 1].T @ ST_h   [65, 2048]    row 64 = softmax denominator l
  y_h   = ytaug_h[:64] * recip(l)  (DVE reciprocal + gpsimd partition bcast)
  AllGather y_h halves over the group -> [256, 1024] x2
  out  += gathered.T @ wo_h      (wo host-permuted to the gathered row order)

v2 notes (vs the 357us baseline): the PE spent the whole baseline run at
the 1.2 GHz mid p-state because the HAM clock gate never saw 3.4us of
continuous matmul work (32 serialized input-DMA issues at the start,
exp-gated micro-gaps in steady state).  This version batches input DMAs
(5 instructions), issues the projection as one dense back-to-back burst
to warm the PE, keeps matmuls flowing through every AllGather window by
finalizing heads in t-halves, and moves softmax-denominator work off the
ACT engine (DVE reciprocal + gpsimd broadcast) since ACT exp is the
steady-state pacer.
"""

import os
import sys

sys.path.insert(0, "/opt/trn_rl_repo")

import numpy as np
import ml_dtypes

import concourse.bass as bass
import concourse.mybir as mybir
import concourse.tile as tile_mod
from concourse.tile import TileContext
from concourse.vector_clock import ScopedClock

F32 = mybir.dt.float32
BF16 = mybir.dt.bfloat16
AF = mybir.ActivationFunctionType

B, T, C = 2, 2048, 1024
H, D = 16, 64
NCORES = 8
GROUP = 4            # cores per batch group (tensor-parallel degree)
HPC = H // GROUP     # heads per core = 4
FQK = 2 * HPC * D    # 512 qk features per core
FV = HPC * D         # 256 v features per core
KT = C // 128        # 8 contraction tiles
TT128 = T // 128     # 16 seq tiles of 128
TT512 = T // 512     # 4 seq tiles of 512
T2 = T // 2          # 1024: finalize/AllGather half
REPLICA_GROUPS = [[0, 1, 2, 3], [4, 5, 6, 7]]

# ---------------------------------------------------------------------------
# walrus workarounds: this build rejects >1 sync-wait command per
# instruction. Move excess waits onto standalone event-semaphore nops on the
# same engine queue (equivalent to raw-bass wait_ge + op).
# ---------------------------------------------------------------------------
_WAITSPLIT_CTR = [0]


def _split_excess_waits(nc: bass.Bass, limit: int = 1) -> int:
    moved = 0
    for f in nc.m.functions:
        for bb in f.blocks:
            insts = bb.instructions
            i = 0
            while i < len(insts):
                inst = insts[i]
                si = inst.sync_info
                if si is not None and si.on_wait and len(si.on_wait) > limit:
                    waits = list(si.on_wait)
                    si.on_wait = waits[:limit]
                    for w in waits[limit:]:
                        _WAITSPLIT_CTR[0] += 1
                        moved += 1
                        ev = mybir.InstEventSemaphore(
                            name=f"I-waitsplit-{_WAITSPLIT_CTR[0]}",
                            engine=inst.engine,
                            ins=[],
                            outs=[],
                            sync_info=mybir.SyncInfo(on_wait=[w], on_update=[]),
                        )
                        insts.insert(i, ev)
                        i += 1
                i += 1
    return moved


def _patched_drain_and_barrier(self, tick_clock, wait_clock):
    nc = self.nc
    nop0 = nc.sync.nop(nofuse=True, hint="tile_exit_waits")
    wait_clock.add_sem_waits(nop0.ins, ScopedClock({None: tick_clock.global_clock}))
    nc.sync.drain()
    nc.all_engine_barrier()
    assert self.sems is not None
    popped = nc._tile_sem_poison_stack.pop()
    assert popped is self._sem_poison
    nc.clear_and_free_semaphores(list(self.sems.allocated().values()))
    nc.all_engine_barrier()


def _install_ntff_shim():
    """Provide antenv.axon_hooks (absent in this image) so trace=True can
    reach the libaxon NTFF profiler."""
    import types

    if "antenv.axon_hooks" in sys.modules:
        return
    hook = None
    try:
        sys.path.insert(0, "/root/.axon_site")
        from trn_agent_boot.trn_boot import _ntff_profile_via_ctypes

        so_path = "/opt/axon/libaxon_pjrt.so"
        if os.path.exists(so_path):
            hook = _ntff_profile_via_ctypes(so_path)
    except Exception:
        hook = None
    mod = types.ModuleType("antenv.axon_hooks")
    mod.get_axon_ntff_profile_hook = lambda: hook
    mod.set_axon_ntff_profile_hook = lambda h: None
    sys.modules["antenv.axon_hooks"] = mod


tile_mod.TileContext._drain_and_barrier = _patched_drain_and_barrier
_install_ntff_shim()


# ---------------------------------------------------------------------------
# device program (identical on all 8 cores; per-core data differs)
# ---------------------------------------------------------------------------
def _build() -> bass.Bass:
    nc = bass.Bass(trn_type="TRN2", target_bir_lowering=False, num_devices=NCORES)

    xT = nc.dram_tensor("xT", [C, T], BF16, kind="ExternalInput")
    wqk = nc.dram_tensor("wqk", [C, FQK], BF16, kind="ExternalInput")
    wv = nc.dram_tensor("wv", [C, FV], BF16, kind="ExternalInput")
    wo = nc.dram_tensor("wo", [FV, HPC * FV], BF16, kind="ExternalInput")
    out = nc.dram_tensor("out", [T, FV], F32, kind="ExternalOutput")

    cc_in = [
        [nc.dram_tensor(f"cc_in{h}_{x2}", [D, T2], BF16) for x2 in range(2)]
        for h in range(HPC)
    ]
    cc_out = [
        [nc.dram_tensor(f"cc_out{h}_{x2}", [GROUP * D, T2], BF16) for x2 in range(2)]
        for h in range(HPC)
    ]

    out_v = out.rearrange("(tt p) f -> tt p f", p=128)

    with TileContext(nc) as tc:
        with (
            tc.tile_pool(name="pw", bufs=1) as pw,
            tc.tile_pool(name="px", bufs=1) as px,
            tc.tile_pool(name="pqkv", bufs=1) as pqkv,
            tc.tile_pool(name="pacc", bufs=1) as pacc,
            tc.tile_pool(name="patt", bufs=2) as patt,
            tc.tile_pool(name="pst", bufs=6) as pst,
            tc.tile_pool(name="pfin", bufs=2) as pfin,
            tc.tile_pool(name="pych", bufs=4) as pych,
            tc.tile_pool(name="ps_big", bufs=2, space="PSUM") as ps_big,
            tc.tile_pool(name="ps_sm", bufs=2, space="PSUM") as ps_sm,
            tc.tile_pool(name="ps_yt", bufs=1, space="PSUM") as ps_yt,
        ):
            # ---- batched input DMAs (5 issues total) -----------------------
            wqk_sb = pw.tile([128, KT * FQK], BF16, name="wqk_sb")
            wv_sb = pw.tile([128, KT * FV], BF16, name="wv_sb")
            wo_sb = pw.tile([128, 2 * HPC * FV], BF16, name="wo_sb")
            x_sb = [px.tile([128, 4 * T], BF16, name=f"x{i}") for i in range(2)]

            nc.sync.dma_start(
                out=wqk_sb[:].rearrange("p (kt f) -> p kt f", f=FQK),
                in_=wqk.rearrange("(kt p) f -> p kt f", p=128),
            )
            xT_v = xT.rearrange("(kt p) t -> p kt t", p=128)
            for i in range(2):
                nc.sync.dma_start(
                    out=x_sb[i][:].rearrange("p (kt t) -> p kt t", t=T),
                    in_=xT_v[:, 4 * i : 4 * (i + 1), :],
                )
            nc.sync.dma_start(
                out=wv_sb[:].rearrange("p (kt f) -> p kt f", f=FV),
                in_=wv.rearrange("(kt p) f -> p kt f", p=128),
            )
            nc.sync.dma_start(
                out=wo_sb[:].rearrange("p (i f) -> p i f", f=HPC * FV),
                in_=wo.rearrange("(i p) f -> p i f", p=128),
            )

            def xs(k, lo, hi):
                return x_sb[k // 4][:, (k % 4) * T + lo : (k % 4) * T + hi]

            ones1 = pw.tile([1, 64], BF16, name="ones1")
            nc.vector.memset(ones1[:], 1.0)

            # persistent activation tiles
            qk_sb = [pqkv.tile([128, T], BF16, name=f"qk{m}") for m in range(4)]
            v_sb = [
                pqkv.tile([128, HPC * (D + 1)], BF16, name=f"v{t}")
                for t in range(TT128)
            ]
            out_acc = [pacc.tile([128, FV], F32, name=f"oacc{t}") for t in range(TT128)]

            # ---- q01 / k01 projection: one dense back-to-back burst -------
            # (wqk dram columns are [q 0..255 | k 0..255] so m=0 -> q heads
            # 01, m=2 -> k heads 01, m=1 -> q23, m=3 -> k23)
            for dst, m in ((0, 0), (1, 2)):
                for q in range(TT512):
                    ps = ps_sm.tile([128, 512], F32, name="sm_ps", tag="sm")
                    for k in range(KT):
                        nc.tensor.matmul(
                            ps[:],
                            wqk_sb[:, k * FQK + 128 * m : k * FQK + 128 * (m + 1)],
                            xs(k, 512 * q, 512 * (q + 1)),
                            start=(k == 0),
                            stop=(k == KT - 1),
                        )
                    nc.vector.tensor_copy(
                        out=qk_sb[dst][:, 512 * q : 512 * (q + 1)], in_=ps[:]
                    )

            # ---- v projection ([t-part, f-col] + ones cols) ----------------
            for t in range(TT128):
                ps = ps_sm.tile([128, 512], F32, name="sm_ps", tag="sm")
                for k in range(KT):
                    nc.tensor.matmul(
                        ps[:, 0:FV],
                        xs(k, 128 * t, 128 * (t + 1)),
                        wv_sb[:, k * FV : (k + 1) * FV],
                        start=(k == 0),
                        stop=(k == KT - 1),
                    )
                vt = v_sb[t].rearrange("p (h g) -> p h g", g=D + 1)
                nc.vector.memset(v_sb[t][:], 1.0)
                nc.vector.tensor_copy(
                    out=vt[:, :, 0:D],
                    in_=ps[:, 0:FV].rearrange("p (h f) -> p h f", f=D),
                )

            # ---- q23 / k23 projection (512-wide, fills attention slack) ----
            for dst, m in ((2, 1), (3, 3)):
                for q in range(TT512):
                    ps = ps_sm.tile([128, 512], F32, name="sm_ps", tag="sm")
                    for k in range(KT):
                        nc.tensor.matmul(
                            ps[:],
                            wqk_sb[:, k * FQK + 128 * m : k * FQK + 128 * (m + 1)],
                            xs(k, 512 * q, 512 * (q + 1)),
                            start=(k == 0),
                            stop=(k == KT - 1),
                        )
                    nc.vector.tensor_copy(
                        out=qk_sb[dst][:, 512 * q : 512 * (q + 1)], in_=ps[:]
                    )

            # ---- attention + per-half finalize / AllGather / out-proj ------
            # The out-projection for a finished t-half is EMITTED one
            # half-phase later than its AllGather is issued: per-engine
            # instruction order is program order, so matmuls that wait on a
            # collective roundtrip must sit behind ~35us of attention work or
            # they head-of-line block the PE queue.
            deferred_op = []

            def emit_outproj(j, x2, ych):
                for hi in range(2):
                    h = 2 * j + hi
                    for tt in range(8):
                        t = 8 * x2 + tt
                        op = ps_sm.tile([128, 512], F32, name="sm_ps", tag="sm")
                        for i in range(2):
                            nc.tensor.matmul(
                                op[:, 0:FV],
                                ych[hi][i][:, 128 * tt : 128 * (tt + 1)],
                                wo_sb[:, HPC * FV * i + FV * h : HPC * FV * i + FV * (h + 1)],
                                start=(i == 0),
                                stop=(i == 1),
                            )
                        if h == 0:
                            nc.vector.tensor_copy(out=out_acc[t][:], in_=op[:, 0:FV])
                        else:
                            nc.vector.tensor_tensor(
                                out=out_acc[t][:],
                                in0=out_acc[t][:],
                                in1=op[:, 0:FV],
                                op=mybir.AluOpType.add,
                            )
                        if h == HPC - 1:
                            nc.sync.dma_start(out=out_v[t], in_=out_acc[t][:])

            for j in range(HPC // 2):  # head pairs (local heads 2j, 2j+1)
                qtile = 2 * j
                ktile = 2 * j + 1
                yt_sb = {
                    hi: patt.tile([D, T], F32, name=f"yt_sb{hi}", tag=f"yt_sb{hi}")
                    for hi in range(2)
                }
                l_sb = {
                    hi: patt.tile([1, T], BF16, name=f"l_sb{hi}", tag=f"l_sb{hi}")
                    for hi in range(2)
                }
                for n in range(TT512):
                    tsl = slice(512 * n, 512 * (n + 1))
                    yt_ps = {
                        hi: ps_yt.tile([D + 1, 512], F32, name=f"yt{hi}", tag=f"yt{hi}")
                        for hi in range(2)
                    }
                    for s in range(TT128):
                        ssl = slice(128 * s, 128 * (s + 1))
                        st_ps = ps_big.tile([128, T2], F32, name="st_ps", tag="big")
                        for hi in range(2):
                            psl = slice(64 * hi, 64 * (hi + 1))
                            nc.tensor.matmul(
                                st_ps[:, 512 * hi : 512 * (hi + 1)],
                                qk_sb[ktile][psl, ssl],
                                qk_sb[qtile][psl, tsl],
                                start=True,
                                stop=True,
                            )
                        ste = pst.tile([128, T2], BF16, name="st_e")
                        nc.scalar.activation(
                            out=ste[:], in_=st_ps[:], func=AF.Exp, scale=0.125
                        )
                        for hi in range(2):
                            h = 2 * j + hi
                            vsl = slice((D + 1) * h, (D + 1) * (h + 1))
                            nc.tensor.matmul(
                                yt_ps[hi][:],
                                v_sb[s][:, vsl],
                                ste[:, 512 * hi : 512 * (hi + 1)],
                                start=(s == 0),
                                stop=(s == TT128 - 1),
                            )
                    for hi in range(2):
                        nc.vector.tensor_copy(
                            out=yt_sb[hi][:, tsl], in_=yt_ps[hi][0:D, :]
                        )
                        nc.vector.tensor_copy(
                            out=l_sb[hi][:, tsl], in_=yt_ps[hi][D : D + 1, :]
                        )

                    if n % 2 == 0:
                        continue
                    # finalize the completed t-half of both heads: normalize
                    # and issue the AllGather now; defer the out-projection
                    x2 = n // 2
                    hsl = slice(T2 * x2, T2 * (x2 + 1))
                    ych = {}
                    for hi in range(2):
                        h = 2 * j + hi
                        # r = 1/l via exp(-ln(l)) on ACT: DVE reciprocal on a
                        # [1,N] AP is single-lane (~8us) and custom-DVE /
                        # gpsimd ISA ops are rejected by this walrus build.
                        lnl = pfin.tile([1, T2], F32, name="lnl", tag="lnl")
                        nc.scalar.activation(
                            out=lnl[:], in_=l_sb[hi][:, hsl], func=AF.Ln
                        )
                        r_h = pfin.tile([1, T2], BF16, name="r_h", tag="r_h")
                        nc.scalar.activation(
                            out=r_h[:], in_=lnl[:], func=AF.Exp, scale=-1.0
                        )
                        ytn = pfin.tile([D, T2], BF16, name="ytn", tag="ytn")
                        for q in range(2):
                            qsl = slice(512 * q, 512 * (q + 1))
                            rb = ps_sm.tile([128, 512], F32, name="sm_ps", tag="sm")
                            nc.tensor.matmul(
                                rb[0:D, :],
                                ones1[:],
                                r_h[:, qsl],
                                start=True,
                                stop=True,
                            )
                            nc.vector.tensor_tensor(
                                out=ytn[:, qsl],
                                in0=yt_sb[hi][0:D, T2 * x2 + 512 * q : T2 * x2 + 512 * (q + 1)],
                                in1=rb[0:D, :],
                                op=mybir.AluOpType.mult,
                            )
                        nc.sync.dma_start(out=cc_in[h][x2][:], in_=ytn[:])
                        nc.gpsimd.collective_compute(
                            "AllGather",
                            mybir.AluOpType.bypass,
                            ins=[cc_in[h][x2][:]],
                            outs=[cc_out[h][x2][:]],
                            replica_groups=REPLICA_GROUPS,
                        )
                        ych[hi] = [
                            pych.tile([128, T2], BF16, name=f"ych{i}", tag=f"ych{i}")
                            for i in range(2)
                        ]
                        for i in range(2):
                            nc.sync.dma_start(
                                out=ych[hi][i][:],
                                in_=cc_out[h][x2][128 * i : 128 * (i + 1), :],
                            )
                    deferred_op.append((j, x2, ych))
                    if len(deferred_op) > 1:
                        emit_outproj(*deferred_op.pop(0))
            while deferred_op:
                emit_outproj(*deferred_op.pop(0))

    _split_excess_waits(nc)
    return nc


_NC_CACHE = []
LAST_RESULTS = None


def kernel(**inputs: np.ndarray) -> np.ndarray:
    global LAST_RESULTS
    from concourse.bass_utils import run_bass_kernel_spmd

    x = np.asarray(inputs["x"], dtype=np.float32)
    W_qkv = np.asarray(inputs["W_qkv"], dtype=np.float32)
    W_out = np.asarray(inputs["W_out"], dtype=np.float32)

    in_maps = []
    for c in range(NCORES):
        g, r = divmod(c, GROUP)
        q_rows = W_qkv[FV * r : FV * (r + 1)]
        k_rows = W_qkv[C + FV * r : C + FV * (r + 1)]
        v_rows = W_qkv[2 * C + FV * r : 2 * C + FV * (r + 1)]
        im = {
            "xT": np.ascontiguousarray(x[g].T).astype(ml_dtypes.bfloat16),
            "wqk": np.ascontiguousarray(
                np.concatenate([q_rows, k_rows], axis=0).T
            ).astype(ml_dtypes.bfloat16),
            "wv": np.ascontiguousarray(v_rows.T).astype(ml_dtypes.bfloat16),
        }
        wo_slice = W_out[FV * r : FV * (r + 1)]  # [256 o, 1024 c]
        wo_heads = []
        for h in range(HPC):
            cols = np.concatenate(
                [np.arange(64 * (GROUP * rr + h), 64 * (GROUP * rr + h) + 64)
                 for rr in range(GROUP)]
            )
            wo_heads.append(wo_slice[:, cols].T)  # [256 c-rows, 256 o]
        im["wo"] = np.ascontiguousarray(np.concatenate(wo_heads, axis=1)).astype(
            ml_dtypes.bfloat16
        )
        in_maps.append(im)

    if not _NC_CACHE:
        _NC_CACHE.append(_build())
    nc = _NC_CACHE[0]

    trace = os.environ.get("KERNEL_TRACE", "0") == "1"
    trace_cores = None
    if trace:
        tc_env = os.environ.get("KERNEL_TRACE_CORES", "0")
        trace_cores = [int(t) for t in tc_env.split(",")]
    res = run_bass_kernel_spmd(
        nc,
        in_maps,
        core_ids=list(range(NCORES)),
        trace=trace,
        trace_cores=trace_cores,
    )
    LAST_RESULTS = res

    out = np.empty((B, T, C), dtype=np.float32)
    for c in range(NCORES):
        g, r = divmod(c, GROUP)
        out[g, :, FV * r : FV * (r + 1)] = res.results[c]["out"]
    return out


# revision 19
# speedup vs baseline: 1.2552x; 1.0249x over previous
"""DDiT attention block on 8 trn2 NeuronCores.

Sharding: data-parallel over batch (cores 0-3 -> batch 0, cores 4-7 ->
batch 1) x tensor-parallel over heads (4 heads/core, Megatron-style:
W_qkv row-sharded, W_out column-sharded). Per-head y shards are
AllGather'd within each 4-core group in t-halves as soon as each half
finishes, and the output projection accumulates per-head chunks, so
collectives overlap the remaining attention compute. Each core produces
a 256-column slice of the output, assembled on the host.

Per core (1 batch, 4 heads, T=2048, C=1024, D=64):
  qT,kT = Wqk_shard @ x.T        [512, 2048]   (features on partitions)
  v     = x @ Wv_shard.T         [2048, 256]   (seq on partitions) + ones col
  ST_h  = exp((kT_h.T @ qT_h)/8) [2048s, 2048t] streamed in [128,512] tiles,
          the two heads of a pair computed as concurrent row-tiled matmuls
  ytaug_h = [v_h | 1].T @ ST_h   [65, 2048]    row 64 = softmax denominator l
  y_h   = ytaug_h[:64] * recip(l)  (DVE reciprocal + gpsimd partition bcast)
  AllGather y_h halves over the group -> [256, 1024] x2
  out  += gathered.T @ wo_h      (wo host-permuted to the gathered row order)

v2 notes (vs the 357us baseline): the PE spent the whole baseline run at
the 1.2 GHz mid p-state because the HAM clock gate never saw 3.4us of
continuous matmul work (32 serialized input-DMA issues at the start,
exp-gated micro-gaps in steady state).  This version batches input DMAs
(5 instructions), issues the projection as one dense back-to-back burst
to warm the PE, keeps matmuls flowing through every AllGather window by
finalizing heads in t-halves, and moves softmax-denominator work off the
ACT engine (DVE reciprocal + gpsimd broadcast) since ACT exp is the
steady-state pacer.
"""

import os
import sys

sys.path.insert(0, "/opt/trn_rl_repo")

import numpy as np
import ml_dtypes

import concourse.bass as bass
import concourse.mybir as mybir
import concourse.tile as tile_mod
from concourse.tile import TileContext
from concourse.vector_clock import ScopedClock

F32 = mybir.dt.float32
BF16 = mybir.dt.bfloat16
AF = mybir.ActivationFunctionType

B, T, C = 2, 2048, 1024
H, D = 16, 64
NCORES = 8
GROUP = 4            # cores per batch group (tensor-parallel degree)
HPC = H // GROUP     # heads per core = 4
FQK = 2 * HPC * D    # 512 qk features per core
FV = HPC * D         # 256 v features per core
KT = C // 128        # 8 contraction tiles
TT128 = T // 128     # 16 seq tiles of 128
TT512 = T // 512     # 4 seq tiles of 512
T2 = T // 2          # 1024: finalize/AllGather half
REPLICA_GROUPS = [[0, 1, 2, 3], [4, 5, 6, 7]]

# ---------------------------------------------------------------------------
# walrus workarounds: this build rejects >1 sync-wait command per
# instruction. Move excess waits onto standalone event-semaphore nops on the
# same engine queue (equivalent to raw-bass wait_ge + op).
# ---------------------------------------------------------------------------
_WAITSPLIT_CTR = [0]


def _split_excess_waits(nc: bass.Bass, limit: int = 1) -> int:
    moved = 0
    for f in nc.m.functions:
        for bb in f.blocks:
            insts = bb.instructions
            i = 0
            while i < len(insts):
                inst = insts[i]
                si = inst.sync_info
                if si is not None and si.on_wait and len(si.on_wait) > limit:
                    waits = list(si.on_wait)
                    si.on_wait = waits[:limit]
                    for w in waits[limit:]:
                        _WAITSPLIT_CTR[0] += 1
                        moved += 1
                        ev = mybir.InstEventSemaphore(
                            name=f"I-waitsplit-{_WAITSPLIT_CTR[0]}",
                            engine=inst.engine,
                            ins=[],
                            outs=[],
                            sync_info=mybir.SyncInfo(on_wait=[w], on_update=[]),
                        )
                        insts.insert(i, ev)
                        i += 1
                i += 1
    return moved


def _patched_drain_and_barrier(self, tick_clock, wait_clock):
    nc = self.nc
    nop0 = nc.sync.nop(nofuse=True, hint="tile_exit_waits")
    wait_clock.add_sem_waits(nop0.ins, ScopedClock({None: tick_clock.global_clock}))
    nc.sync.drain()
    nc.all_engine_barrier()
    assert self.sems is not None
    popped = nc._tile_sem_poison_stack.pop()
    assert popped is self._sem_poison
    nc.clear_and_free_semaphores(list(self.sems.allocated().values()))
    nc.all_engine_barrier()


def _install_ntff_shim():
    """Provide antenv.axon_hooks (absent in this image) so trace=True can
    reach the libaxon NTFF profiler."""
    import types

    if "antenv.axon_hooks" in sys.modules:
        return
    hook = None
    try:
        sys.path.insert(0, "/root/.axon_site")
        from trn_agent_boot.trn_boot import _ntff_profile_via_ctypes

        so_path = "/opt/axon/libaxon_pjrt.so"
        if os.path.exists(so_path):
            hook = _ntff_profile_via_ctypes(so_path)
    except Exception:
        hook = None
    mod = types.ModuleType("antenv.axon_hooks")
    mod.get_axon_ntff_profile_hook = lambda: hook
    mod.set_axon_ntff_profile_hook = lambda h: None
    sys.modules["antenv.axon_hooks"] = mod


tile_mod.TileContext._drain_and_barrier = _patched_drain_and_barrier
_install_ntff_shim()


# ---------------------------------------------------------------------------
# device program (identical on all 8 cores; per-core data differs)
# ---------------------------------------------------------------------------
def _build() -> bass.Bass:
    nc = bass.Bass(trn_type="TRN2", target_bir_lowering=False, num_devices=NCORES)

    xT = nc.dram_tensor("xT", [C, T], BF16, kind="ExternalInput")
    wqk = nc.dram_tensor("wqk", [C, FQK], BF16, kind="ExternalInput")
    wv = nc.dram_tensor("wv", [C, FV], BF16, kind="ExternalInput")
    wo = nc.dram_tensor("wo", [HPC * FV, FV], BF16, kind="ExternalInput")
    out = nc.dram_tensor("out", [T, FV], F32, kind="ExternalOutput")

    cc_in = [
        [nc.dram_tensor(f"cc_in{j}_{x2}", [128, T2], BF16) for x2 in range(2)]
        for j in range(2)
    ]
    cc_out = [
        [nc.dram_tensor(f"cc_out{j}_{x2}", [GROUP * 128, T2], BF16) for x2 in range(2)]
        for j in range(2)
    ]

    out_v = out.rearrange("(tt p) f -> tt p f", p=128)

    with TileContext(nc) as tc:
        with (
            tc.tile_pool(name="pw", bufs=1) as pw,
            tc.tile_pool(name="px", bufs=1) as px,
            tc.tile_pool(name="pqkv", bufs=1) as pqkv,
            tc.tile_pool(name="pacc", bufs=1) as pacc,
            tc.tile_pool(name="patt", bufs=2) as patt,
            tc.tile_pool(name="pst", bufs=6) as pst,
            tc.tile_pool(name="pfin", bufs=2) as pfin,
            tc.tile_pool(name="pych", bufs=4) as pych,
            tc.tile_pool(name="ps_big", bufs=2, space="PSUM") as ps_big,
            tc.tile_pool(name="ps_sm", bufs=2, space="PSUM") as ps_sm,
            tc.tile_pool(name="ps_yt", bufs=1, space="PSUM") as ps_yt,
        ):
            # ---- batched input DMAs (5 issues total) -----------------------
            wqk_sb = pw.tile([128, KT * FQK], BF16, name="wqk_sb")
            wv_sb = pw.tile([128, KT * FV], BF16, name="wv_sb")
            wo_sb = pw.tile([128, 2 * HPC * FV], BF16, name="wo_sb")
            x_sb = [px.tile([128, 4 * T], BF16, name=f"x{i}") for i in range(2)]

            nc.sync.dma_start(
                out=wqk_sb[:].rearrange("p (kt f) -> p kt f", f=FQK),
                in_=wqk.rearrange("(kt p) f -> p kt f", p=128),
            )
            xT_v = xT.rearrange("(kt p) t -> p kt t", p=128)
            for i in range(2):
                nc.sync.dma_start(
                    out=x_sb[i][:].rearrange("p (kt t) -> p kt t", t=T),
                    in_=xT_v[:, 4 * i : 4 * (i + 1), :],
                )
            nc.sync.dma_start(
                out=wv_sb[:].rearrange("p (kt f) -> p kt f", f=FV),
                in_=wv.rearrange("(kt p) f -> p kt f", p=128),
            )
            nc.sync.dma_start(
                out=wo_sb[:].rearrange("p (c f) -> p c f", f=FV),
                in_=wo.rearrange("(c p) f -> p c f", p=128),
            )

            def xs(k, lo, hi):
                return x_sb[k // 4][:, (k % 4) * T + lo : (k % 4) * T + hi]

            ones1 = pw.tile([1, 64], BF16, name="ones1")
            nc.vector.memset(ones1[:], 1.0)

            # persistent activation tiles
            qk_sb = [pqkv.tile([128, T], BF16, name=f"qk{m}") for m in range(4)]
            v_sb = [
                pqkv.tile([128, HPC * (D + 1)], BF16, name=f"v{t}")
                for t in range(TT128)
            ]
            out_acc = [pacc.tile([128, FV], F32, name=f"oacc{t}") for t in range(TT128)]

            # ---- q01 / k01 projection: one dense back-to-back burst -------
            # (wqk dram columns are [q 0..255 | k 0..255] so m=0 -> q heads
            # 01, m=2 -> k heads 01, m=1 -> q23, m=3 -> k23)
            for dst, m in ((0, 0), (1, 2)):
                for q in range(TT512):
                    ps = ps_sm.tile([128, 512], F32, name="sm_ps", tag="sm")
                    for k in range(KT):
                        nc.tensor.matmul(
                            ps[:],
                            wqk_sb[:, k * FQK + 128 * m : k * FQK + 128 * (m + 1)],
                            xs(k, 512 * q, 512 * (q + 1)),
                            start=(k == 0),
                            stop=(k == KT - 1),
                        )
                    nc.vector.tensor_copy(
                        out=qk_sb[dst][:, 512 * q : 512 * (q + 1)], in_=ps[:]
                    )

            # ---- v projection ([t-part, f-col] + ones cols) ----------------
            for t in range(TT128):
                ps = ps_sm.tile([128, 512], F32, name="sm_ps", tag="sm")
                for k in range(KT):
                    nc.tensor.matmul(
                        ps[:, 0:FV],
                        xs(k, 128 * t, 128 * (t + 1)),
                        wv_sb[:, k * FV : (k + 1) * FV],
                        start=(k == 0),
                        stop=(k == KT - 1),
                    )
                vt = v_sb[t].rearrange("p (h g) -> p h g", g=D + 1)
                nc.vector.memset(v_sb[t][:], 1.0)
                nc.vector.tensor_copy(
                    out=vt[:, :, 0:D],
                    in_=ps[:, 0:FV].rearrange("p (h f) -> p h f", f=D),
                )

            # ---- q23 / k23 projection (512-wide, fills attention slack) ----
            for dst, m in ((2, 1), (3, 3)):
                for q in range(TT512):
                    ps = ps_sm.tile([128, 512], F32, name="sm_ps", tag="sm")
                    for k in range(KT):
                        nc.tensor.matmul(
                            ps[:],
                            wqk_sb[:, k * FQK + 128 * m : k * FQK + 128 * (m + 1)],
                            xs(k, 512 * q, 512 * (q + 1)),
                            start=(k == 0),
                            stop=(k == KT - 1),
                        )
                    nc.vector.tensor_copy(
                        out=qk_sb[dst][:, 512 * q : 512 * (q + 1)], in_=ps[:]
                    )

            # ---- attention + per-half finalize / AllGather / out-proj ------
            # The out-projection for a finished t-half is EMITTED one
            # half-phase later than its AllGather is issued: per-engine
            # instruction order is program order, so matmuls that wait on a
            # collective roundtrip must sit behind ~35us of attention work or
            # they head-of-line block the PE queue.
            deferred_op = []

            def emit_outproj(j, x2, ych):
                for tt in range(8):
                    t = 8 * x2 + tt
                    op = ps_sm.tile([128, 512], F32, name="sm_ps", tag="sm")
                    for rr in range(GROUP):
                        nc.tensor.matmul(
                            op[:, 0:FV],
                            ych[rr][:, 128 * tt : 128 * (tt + 1)],
                            wo_sb[:, FV * (GROUP * j + rr) : FV * (GROUP * j + rr + 1)],
                            start=(rr == 0),
                            stop=(rr == GROUP - 1),
                        )
                    if j == 0:
                        nc.vector.tensor_copy(out=out_acc[t][:], in_=op[:, 0:FV])
                    else:
                        nc.vector.tensor_tensor(
                            out=out_acc[t][:],
                            in0=out_acc[t][:],
                            in1=op[:, 0:FV],
                            op=mybir.AluOpType.add,
                        )
                        nc.sync.dma_start(out=out_v[t], in_=out_acc[t][:])

            for j in range(HPC // 2):  # head pairs (local heads 2j, 2j+1)
                qtile = 2 * j
                ktile = 2 * j + 1
                yt_sb = {
                    hi: patt.tile([D, T], F32, name=f"yt_sb{hi}", tag=f"yt_sb{hi}")
                    for hi in range(2)
                }
                l_sb = {
                    hi: patt.tile([1, T], BF16, name=f"l_sb{hi}", tag=f"l_sb{hi}")
                    for hi in range(2)
                }
                for n in range(TT512):
                    tsl = slice(512 * n, 512 * (n + 1))
                    yt_ps = {
                        hi: ps_yt.tile([D + 1, 512], F32, name=f"yt{hi}", tag=f"yt{hi}")
                        for hi in range(2)
                    }
                    for s in range(TT128):
                        ssl = slice(128 * s, 128 * (s + 1))
                        st_ps = ps_big.tile([128, T2], F32, name="st_ps", tag="big")
                        for hi in range(2):
                            psl = slice(64 * hi, 64 * (hi + 1))
                            nc.tensor.matmul(
                                st_ps[:, 512 * hi : 512 * (hi + 1)],
                                qk_sb[ktile][psl, ssl],
                                qk_sb[qtile][psl, tsl],
                                start=True,
                                stop=True,
                            )
                        ste = pst.tile([128, T2], BF16, name="st_e")
                        nc.scalar.activation(
                            out=ste[:], in_=st_ps[:], func=AF.Exp, scale=0.125
                        )
                        for hi in range(2):
                            h = 2 * j + hi
                            vsl = slice((D + 1) * h, (D + 1) * (h + 1))
                            nc.tensor.matmul(
                                yt_ps[hi][:],
                                v_sb[s][:, vsl],
                                ste[:, 512 * hi : 512 * (hi + 1)],
                                start=(s == 0),
                                stop=(s == TT128 - 1),
                            )
                    for hi in range(2):
                        nc.vector.tensor_copy(
                            out=yt_sb[hi][:, tsl], in_=yt_ps[hi][0:D, :]
                        )
                        nc.vector.tensor_copy(
                            out=l_sb[hi][:, tsl], in_=yt_ps[hi][D : D + 1, :]
                        )

                    if n % 2 == 0:
                        continue
                    # finalize the completed t-half of both heads: normalize
                    # and issue the AllGather now; defer the out-projection
                    x2 = n // 2
                    hsl = slice(T2 * x2, T2 * (x2 + 1))
                    # both heads of the pair share one [128, T2] gather
                    ytn = pfin.tile([128, T2], BF16, name="ytn", tag="ytn")
                    for hi in range(2):
                        # r = 1/l via exp(-ln(l)) on ACT: DVE reciprocal on a
                        # [1,N] AP is single-lane (~8us) and custom-DVE /
                        # gpsimd ISA ops are rejected by this walrus build.
                        lnl = pfin.tile([1, T2], F32, name="lnl", tag="lnl")
                        nc.scalar.activation(
                            out=lnl[:], in_=l_sb[hi][:, hsl], func=AF.Ln
                        )
                        r_h = pfin.tile([1, T2], BF16, name="r_h", tag="r_h")
                        nc.scalar.activation(
                            out=r_h[:], in_=lnl[:], func=AF.Exp, scale=-1.0
                        )
                        for q in range(2):
                            qsl = slice(512 * q, 512 * (q + 1))
                            rb = ps_sm.tile([128, 512], F32, name="sm_ps", tag="sm")
                            nc.tensor.matmul(
                                rb[0:D, :],
                                ones1[:],
                                r_h[:, qsl],
                                start=True,
                                stop=True,
                            )
                            nc.vector.tensor_tensor(
                                out=ytn[D * hi : D * (hi + 1), qsl],
                                in0=yt_sb[hi][0:D, T2 * x2 + 512 * q : T2 * x2 + 512 * (q + 1)],
                                in1=rb[0:D, :],
                                op=mybir.AluOpType.mult,
                            )
                    nc.sync.dma_start(out=cc_in[j][x2][:], in_=ytn[:])
                    nc.gpsimd.collective_compute(
                        "AllGather",
                        mybir.AluOpType.bypass,
                        ins=[cc_in[j][x2][:]],
                        outs=[cc_out[j][x2][:]],
                        replica_groups=REPLICA_GROUPS,
                    )
                    ych = [
                        pych.tile([128, T2], BF16, name=f"ych{rr}", tag=f"ych{rr % 2}")
                        for rr in range(GROUP)
                    ]
                    for rr in range(GROUP):
                        nc.sync.dma_start(
                            out=ych[rr][:],
                            in_=cc_out[j][x2][128 * rr : 128 * (rr + 1), :],
                        )
                    deferred_op.append((j, x2, ych))
                    if len(deferred_op) > 1:
                        emit_outproj(*deferred_op.pop(0))
            while deferred_op:
                emit_outproj(*deferred_op.pop(0))

    _split_excess_waits(nc)
    return nc


_NC_CACHE = []
LAST_RESULTS = None


def kernel(**inputs: np.ndarray) -> np.ndarray:
    global LAST_RESULTS
    from concourse.bass_utils import run_bass_kernel_spmd

    x = np.asarray(inputs["x"], dtype=np.float32)
    W_qkv = np.asarray(inputs["W_qkv"], dtype=np.float32)
    W_out = np.asarray(inputs["W_out"], dtype=np.float32)

    in_maps = []
    for c in range(NCORES):
        g, r = divmod(c, GROUP)
        q_rows = W_qkv[FV * r : FV * (r + 1)]
        k_rows = W_qkv[C + FV * r : C + FV * (r + 1)]
        v_rows = W_qkv[2 * C + FV * r : 2 * C + FV * (r + 1)]
        im = {
            "xT": np.ascontiguousarray(x[g].T).astype(ml_dtypes.bfloat16),
            "wqk": np.ascontiguousarray(
                np.concatenate([q_rows, k_rows], axis=0).T
            ).astype(ml_dtypes.bfloat16),
            "wv": np.ascontiguousarray(v_rows.T).astype(ml_dtypes.bfloat16),
        }
        wo_slice = W_out[FV * r : FV * (r + 1)]  # [256 o, 1024 c]
        # c-row order must match the gathered layout: for pair j, rank rr
        # contributes [head(4rr+2j) | head(4rr+2j+1)] d-blocks
        wo_rows = []
        for j in range(2):
            for rr in range(GROUP):
                for hi in range(2):
                    g = GROUP * rr + 2 * j + hi
                    wo_rows.append(wo_slice[:, 64 * g : 64 * (g + 1)].T)
        im["wo"] = np.ascontiguousarray(np.concatenate(wo_rows, axis=0)).astype(
            ml_dtypes.bfloat16
        )
        in_maps.append(im)

    if not _NC_CACHE:
        _NC_CACHE.append(_build())
    nc = _NC_CACHE[0]

    trace = os.environ.get("KERNEL_TRACE", "0") == "1"
    trace_cores = None
    if trace:
        tc_env = os.environ.get("KERNEL_TRACE_CORES", "0")
        trace_cores = [int(t) for t in tc_env.split(",")]
    res = run_bass_kernel_spmd(
        nc,
        in_maps,
        core_ids=list(range(NCORES)),
        trace=trace,
        trace_cores=trace_cores,
    )
    LAST_RESULTS = res

    out = np.empty((B, T, C), dtype=np.float32)
    for c in range(NCORES):
        g, r = divmod(c, GROUP)
        out[g, :, FV * r : FV * (r + 1)] = res.results[c]["out"]
    return out
